# revision 7
# baseline (speedup 1.0000x reference)
"""Trainium2 Bass kernel for AdaptiveLogSoftmaxWithLoss (moe_routing).

Strategy: tensor-shard the three class dimensions (head 4002->4096 pad,
tail0 16000, tail1 30257->30720 pad) across 8 cores.  Each core:
  - computes hidden projections h0T=[512,2048], h1T=[256,2048] (redundantly,
    in transposed layout so they feed the big GEMMs as lhsT with no on-device
    transposes),
  - computes its logit shard in [sample, class] PSUM chunks,
  - per chunk: ACT exp with accum_out -> partial sum-exp per row (logits are
    tiny by construction, |x| < ~4, so no max subtraction is needed),
    and one fused DVE scalar_tensor_tensor (iota == rel) * logit with
    accum_out -> target-logit partial per row.
Host combines: lse = log(sum over cores of partial sums, minus the exact
pad contribution exp(0)=1 per zero-padded column), gathers are summed
(exactly one core's eq-mask fires), and the reference formula is applied.
All heavy math (GEMMs, exp, reductions, gather) runs on device.
"""

import numpy as np
import ml_dtypes

import concourse.bass as bass
import concourse.bacc as bacc
import concourse.mybir as mybir
import concourse.tile as tile
from concourse.bass_utils import run_bass_kernel_spmd

BF16 = ml_dtypes.bfloat16
NCORES = 8
N, D = 2048, 1024
H0, H1 = 512, 256
C0, C1 = 4000, 20000
HEAD = 4002        # 4000 shortlist + 2 cluster-logit columns
HEAD_PAD = 4096    # padded so 8 cores get 512 each
T0 = 16000
T0_PAD = 16384     # padded so 8 cores get 2048 each
T1 = 30257
T1_PAD = 30720     # padded so 8 cores get 3840 each
WH, W0, W1 = HEAD_PAD // 8, T0_PAD // 8, T1_PAD // 8   # 512, 2048, 3840
MT = N // 128                                          # 16 sample tiles
PAD_H = HEAD_PAD - HEAD   # 94 zero columns, all on core 7
PAD_0 = T0_PAD - T0       # 384 zero columns, all on core 7
PAD_1 = T1_PAD - T1       # 463 zero columns, all on core 7

# module-level knobs for test.py (harness never touches these)
TRACE = False
LAST_RESULT = None

_CACHED_NC = None


def _build_nc():
    nc = bacc.Bacc(None)
    BF = mybir.dt.bfloat16
    F32 = mybir.dt.float32
    AX = mybir.AxisListType
    OP = mybir.AluOpType
    ACTF = mybir.ActivationFunctionType

    inpT_d = nc.dram_tensor("inpT", [128, D // 128, N], BF, kind="ExternalInput")
    w1t0_d = nc.dram_tensor("w1t0", [128, D // 128, H0], BF, kind="ExternalInput")
    w1t1_d = nc.dram_tensor("w1t1", [128, D // 128, H1], BF, kind="ExternalInput")
    hwT_d = nc.dram_tensor("hwT", [128, D // 128, WH], BF, kind="ExternalInput")
    w2t0_d = nc.dram_tensor("w2t0", [128, H0 // 128, W0], BF, kind="ExternalInput")
    w2t1_d = nc.dram_tensor("w2t1", [128, H1 // 128, W1], BF, kind="ExternalInput")
    iota_d = nc.dram_tensor("iota", [128, 4096], F32, kind="ExternalInput")
    rels_d = nc.dram_tensor("rels", [128, MT, 3], F32, kind="ExternalInput")
    res_d = nc.dram_tensor("res", [128, MT, 6], F32, kind="ExternalOutput")

    with tile.TileContext(nc) as tc:
        with (
            tc.tile_pool(name="const", bufs=1) as cp,
            tc.tile_pool(name="work", bufs=3) as wp,
            tc.tile_pool(name="parts", bufs=4) as pp,
            tc.tile_pool(name="psum", bufs=2, space="PSUM") as psp,
        ):
            inpT = cp.tile([128, D // 128, N], BF)
            w1t0 = cp.tile([128, D // 128, H0], BF)
            w1t1 = cp.tile([128, D // 128, H1], BF)
            hwT = cp.tile([128, D // 128, WH], BF)
            w2t0 = cp.tile([128, H0 // 128, W0], BF)
            w2t1 = cp.tile([128, H1 // 128, W1], BF)
            iota = cp.tile([128, 4096], F32)
            rels = cp.tile([128, MT, 3], F32)
            h0T = cp.tile([128, H0 // 128, N], BF)
            h1T = cp.tile([128, H1 // 128, N], BF)
            res = cp.tile([128, MT, 6], F32)

            # loads ordered so phase-1 deps land first
            nc.sync.dma_start(w1t0[:], w1t0_d[:])
            nc.sync.dma_start(w1t1[:], w1t1_d[:])
            nc.sync.dma_start(inpT[:], inpT_d[:])
            nc.sync.dma_start(iota[:], iota_d[:])
            nc.sync.dma_start(rels[:], rels_d[:])
            nc.sync.dma_start(hwT[:], hwT_d[:])
            nc.sync.dma_start(w2t0[:], w2t0_d[:])
            nc.sync.dma_start(w2t1[:], w2t1_d[:])

            # phase 1: hidden projections, transposed layout hT[h, sample]
            with nc.named_scope("hidden"):
                for hT, w1, hdim in ((h0T, w1t0, H0), (h1T, w1t1, H1)):
                    for mh in range(hdim // 128):
                        for rc in range(N // 512):
                            ps = psp.tile([128, 2048], F32, tag="logits")
                            for kt in range(D // 128):
                                nc.tensor.matmul(
                                    ps[:, :512],
                                    w1[:, kt, mh * 128 : (mh + 1) * 128],
                                    inpT[:, kt, rc * 512 : (rc + 1) * 512],
                                    start=(kt == 0),
                                    stop=(kt == D // 128 - 1),
                                )
                            nc.vector.tensor_copy(
                                hT[:, mh, rc * 512 : (rc + 1) * 512], ps[:, :512]
                            )

            # phase 2: sharded logits in wide (multi-bank) PSUM groups; one
            # exp+accum (ACT) and one (iota==rel)*logit+accum (DVE) per group
            def post(ps, w, ioff, rel_ap, s_ap, t_ap):
                sc_e = wp.tile([128, 2048], BF, tag="sc_e")
                sc_t = wp.tile([128, 2048], BF, tag="sc_t")
                nc.scalar.activation(
                    sc_e[:, :w], ps[:, :w], ACTF.Exp, accum_out=s_ap
                )
                nc.vector.scalar_tensor_tensor(
                    out=sc_t[:, :w],
                    in0=iota[:, ioff : ioff + w],
                    scalar=rel_ap,
                    in1=ps[:, :w],
                    op0=OP.is_equal,
                    op1=OP.mult,
                    accum_out=t_ap,
                )

            # (scope, lhsT, w2, K, group width, w2/iota col offset, cluster,
            #  partial slot or None for direct res write)
            groups = [
                ("head", None, None, D, WH, 0, 0, None),
                ("tail0", h0T, w2t0, H0, 2048, 0, 1, None),
                ("tail1", h1T, w2t1, H1, 2048, 0, 2, 0),
                ("tail1", h1T, w2t1, H1, 1792, 2048, 2, 1),
            ]

            for m in range(MT):
                ms = slice(m * 128, (m + 1) * 128)
                spart = pp.tile([128, 2], F32, tag="spart")
                tpart = pp.tile([128, 2], F32, tag="tpart")
                for name, lhsT, w2, kdim, gw, goff, ci, slot in groups:
                    with nc.named_scope(name):
                        ps = psp.tile([128, 2048], F32, tag="logits")
                        for co in range(0, gw, 512):
                            cw = min(512, gw - co)
                            for kt in range(kdim // 128):
                                nc.tensor.matmul(
                                    ps[:, co : co + cw],
                                    inpT[:, kt, ms] if w2 is None else lhsT[:, kt, ms],
                                    hwT[:, kt, :cw] if w2 is None
                                    else w2[:, kt, goff + co : goff + co + cw],
                                    start=(kt == 0),
                                    stop=(kt == kdim // 128 - 1),
                                )
                        if slot is None:
                            s_ap = res[:, m, ci : ci + 1]
                            t_ap = res[:, m, 3 + ci : 4 + ci]
                        else:
                            s_ap = spart[:, slot : slot + 1]
                            t_ap = tpart[:, slot : slot + 1]
                        post(ps, gw, goff, rels[:, m, ci : ci + 1], s_ap, t_ap)
                nc.vector.reduce_sum(res[:, m, 2:3], spart[:, :2], axis=AX.X)
                nc.vector.reduce_sum(res[:, m, 5:6], tpart[:, :2], axis=AX.X)

            nc.sync.dma_start(res_d[:], res[:])

    nc.finalize()
    return nc


def _get_nc():
    global _CACHED_NC
    if _CACHED_NC is None:
        _CACHED_NC = _build_nc()
    return _CACHED_NC


def _tiled(a2d):
    """[K, F] (K multiple of 128) -> contiguous [128, K//128, F]."""
    K, F = a2d.shape
    return np.ascontiguousarray(
        a2d.reshape(K // 128, 128, F).transpose(1, 0, 2)
    )


def _pm(vec):
    """[N] -> [128, MT] with [p, m] = vec[m*128+p]."""
    return np.ascontiguousarray(vec.reshape(MT, 128).T)


def _unpm(a):
    """[128, MT] -> [N]."""
    return np.ascontiguousarray(a.T).reshape(N)


def make_in_maps(inp, tgt, head_w, t0_w1, t0_w2, t1_w1, t1_w2):
    inp = np.asarray(inp, dtype=np.float32)
    tgt = np.asarray(tgt).astype(np.int64)

    inpT = _tiled(inp.T.astype(BF16))
    w1t0 = _tiled(np.asarray(t0_w1, np.float32).T.astype(BF16))
    w1t1 = _tiled(np.asarray(t1_w1, np.float32).T.astype(BF16))

    hwT_full = np.zeros((D, HEAD_PAD), BF16)
    hwT_full[:, :HEAD] = np.asarray(head_w, np.float32).T.astype(BF16)
    w2t0_full = np.zeros((H0, T0_PAD), BF16)
    w2t0_full[:, :T0] = np.asarray(t0_w2, np.float32).T.astype(BF16)
    w2t1_full = np.zeros((H1, T1_PAD), BF16)
    w2t1_full[:, :T1] = np.asarray(t1_w2, np.float32).T.astype(BF16)

    iota = np.broadcast_to(
        np.arange(4096, dtype=np.float32)[None, :], (128, 4096)
    ).copy()

    gi = np.where(tgt < C0, tgt, np.where(tgt < C1, C0, C0 + 1))
    rel0 = tgt - C0
    rel1 = tgt - C1

    in_maps = []
    for i in range(NCORES):
        rels = np.stack(
            [
                _pm((gi - i * WH).astype(np.float32)),
                _pm((rel0 - i * W0).astype(np.float32)),
                _pm((rel1 - i * W1).astype(np.float32)),
            ],
            axis=2,
        )
        in_maps.append(
            {
                "inpT": inpT,
                "w1t0": w1t0,
                "w1t1": w1t1,
                "hwT": _tiled(hwT_full[:, i * WH : (i + 1) * WH]),
                "w2t0": _tiled(w2t0_full[:, i * W0 : (i + 1) * W0]),
                "w2t1": _tiled(w2t1_full[:, i * W1 : (i + 1) * W1]),
                "iota": iota,
                "rels": np.ascontiguousarray(rels),
            }
        )
    return in_maps, tgt


def combine(results, tgt):
    """results: list of per-core {'res': [128, MT, 6]} -> final [N] f32 NLL."""
    S = np.zeros((3, N), np.float64)
    T = np.zeros((3, N), np.float64)
    for r in results:
        res = np.asarray(r["res"], np.float64)
        for c in range(3):
            S[c] += _unpm(res[:, :, c])
            T[c] += _unpm(res[:, :, 3 + c])
    S[0] -= PAD_H  # zero-padded columns contribute exp(0)=1 each (core 7)
    S[1] -= PAD_0
    S[2] -= PAD_1

    in1 = (tgt >= C0) & (tgt < C1)
    in2 = tgt >= C1
    head_term = T[0] - np.log(S[0])
    lp0 = T[1] - np.log(S[1])
    lp1 = T[2] - np.log(S[2])
    out = head_term + np.where(in1, lp0, 0.0) + np.where(in2, lp1, 0.0)
    return (-out).astype(np.float32)


def kernel(inp, tgt, head_w, t0_w1, t0_w2, t1_w1, t1_w2):
    global LAST_RESULT
    nc = _get_nc()
    in_maps, tgt64 = make_in_maps(inp, tgt, head_w, t0_w1, t0_w2, t1_w1, t1_w2)
    out = run_bass_kernel_spmd(
        nc, in_maps, core_ids=list(range(NCORES)), trace=TRACE
    )
    LAST_RESULT = out
    return combine(out.results, tgt64)
